# revision 44
# baseline (speedup 1.0000x reference)
"""Trainium2 Bass kernel for nn_ListenerModel (scatter_memory).

Strategy: pure data-parallel over batch (B=64 -> 8 rows/core), weights
replicated.  Key points:
  - masked sequence positions have softmax weight exactly 0, so the
    host compacts each batch's sequence to its unmasked positions
    (variable per-batch length lp_b, rounded to 8).  Math is exact;
    pad slots get -1e30 mask.  This cuts the dominant matmul chain and
    reps DMA ~2x.
  - all matmul operands bf16 (fp32r streams 2 cyc/col; bf16 1).
  - host lays every tensor out partition-major so DMAs are contiguous;
    two queues (sync: start-critical + activations, gpsimd: W_vis
    stream + late weights) ordered by need-time; reps pool holds all
    8 batches so no DMA ever waits on compute.
  - visual-context matmuls (M=8) packed 4-wide into PE column groups,
    interleaved into mm1 against modeled DMA arrival times.
  - per-b softmax/attend runs several blocks behind the matmul stream;
    sep/history fillers are interleaved between early per-b blocks
    (their data arrives on the queue tails); outputs produced per-b
    via attcol x sepfinT matmuls (no serial transpose tail).
"""

import numpy as np
import ml_dtypes
from contextlib import ExitStack

import concourse.bass as bass
import concourse.mybir as mybir
from concourse import bacc, tile
from concourse.bass_utils import run_bass_kernel_spmd

NCORES = 8
B, L, S, H = 64, 512, 6, 8
EMBED, HID, IMG, ATT = 1024, 512, 2048, 256
SIMG = S * IMG          # 12288
BC = B // NCORES        # 8 batch rows per core
BS = BC * S             # 48 (b,s) rows per core
BSH = BS * H            # 384
P = 128
FP = mybir.dt.float32
BF = mybir.dt.bfloat16

KE = EMBED // P         # 8  k-chunks for EMBED contraction
KH = HID // P           # 4  k-chunks for HID contraction
KA = ATT // P           # 2  k-chunks for ATT contraction
KV = SIMG // P          # 96 k-chunks for the visual-context matmul
KI = IMG // P           # 16 k-chunks for separate-image projection
KBH = BSH // P          # 3  k-chunks for history averaging
NHT = HID // P          # 4  hid tiles
NAT = ATT // P          # 2  att tiles

WVB = 4                 # W_vis chunks per DMA / per packed vc group
NVG = KV // WVB         # 24 vc chunk groups


def build_nc(lps):
    """lps: per-core tuple of BC compacted sequence lengths."""
    lmax = max(lps)
    nc = bacc.Bacc(None)

    d_reps = nc.dram_tensor("repsT", [BC, P, KE * lmax], BF,
                            kind="ExternalInput")
    d_vcT = nc.dram_tensor("vcT", [P, KV * BC], BF, kind="ExternalInput")
    d_sepT = nc.dram_tensor("sepT", [P, KI * BS], BF, kind="ExternalInput")
    d_hist = nc.dram_tensor("histf", [P, KBH * EMBED], BF, kind="ExternalInput")
    d_validW = nc.dram_tensor("validW", [P, KBH * BS], BF, kind="ExternalInput")
    d_Wvis = nc.dram_tensor("Wvis", [NVG, P, WVB * HID], BF, kind="ExternalInput")
    d_Wemb = nc.dram_tensor("Wemb", [P, KE * HID], BF, kind="ExternalInput")
    d_Wmm = nc.dram_tensor("Wmm", [P, 2 * KH * HID], BF, kind="ExternalInput")
    d_Wsep = nc.dram_tensor("Wsep", [P, KI * HID], BF, kind="ExternalInput")
    d_Wa1 = nc.dram_tensor("Wa1", [P, KH * ATT], BF, kind="ExternalInput")
    d_Wa2 = nc.dram_tensor("Wa2", [P, KA], BF, kind="ExternalInput")
    d_bvis = nc.dram_tensor("bvis_row", [1, HID], BF, kind="ExternalInput")
    d_bsep = nc.dram_tensor("bsep_row", [1, HID], BF, kind="ExternalInput")
    d_bemb_row = nc.dram_tensor("bemb_row", [1, HID], BF, kind="ExternalInput")
    d_ones = nc.dram_tensor("ones_row", [1, P], BF, kind="ExternalInput")
    d_bemb_col = nc.dram_tensor("bemb_col", [P, NHT], FP, kind="ExternalInput")
    d_bmm_col = nc.dram_tensor("bmm_col", [P, NHT], FP, kind="ExternalInput")
    d_ba1_col = nc.dram_tensor("ba1_col", [P, NAT], FP, kind="ExternalInput")
    d_mask = nc.dram_tensor("mask_row", [BC, lmax], FP, kind="ExternalInput")
    d_hh = nc.dram_tensor("hh_col", [BS, 1], FP, kind="ExternalInput")
    d_ident = nc.dram_tensor("ident", [P, P], FP, kind="ExternalInput")
    d_gsel = nc.dram_tensor("gsel", [P, BC], BF, kind="ExternalInput")
    d_out = nc.dram_tensor("out", [1, BS], FP, kind="ExternalOutput")

    AFT = mybir.ActivationFunctionType
    AX = mybir.AxisListType

    with ExitStack() as ctx:
        tc = ctx.enter_context(tile.TileContext(nc))
        wres = ctx.enter_context(tc.tile_pool(name="wres", bufs=1))
        repsp = ctx.enter_context(tc.tile_pool(name="repsp", bufs=8))
        wvp = ctx.enter_context(tc.tile_pool(name="wvp", bufs=4))
        wvp2 = ctx.enter_context(tc.tile_pool(name="wvp2", bufs=8))
        mm1p = ctx.enter_context(tc.tile_pool(name="mm1p", bufs=32))
        mm2p = ctx.enter_context(tc.tile_pool(name="mm2p", bufs=20))
        atthp = ctx.enter_context(tc.tile_pool(name="atthp", bufs=4))
        tmpp = ctx.enter_context(tc.tile_pool(name="tmpp", bufs=3))
        wbtp = ctx.enter_context(tc.tile_pool(name="wbtp", bufs=2))
        attcp = ctx.enter_context(tc.tile_pool(name="attcp", bufs=32))
        smp = ctx.enter_context(tc.tile_pool(name="smp", bufs=3))
        wrp = ctx.enter_context(tc.tile_pool(name="wrp", bufs=8))
        # PSUM: 2 + 1 + 2 + 3 = 8 banks
        psM = ctx.enter_context(tc.tile_pool(name="psM", bufs=2, space="PSUM"))
        psV = ctx.enter_context(tc.tile_pool(name="psV", bufs=1, space="PSUM"))
        psE = ctx.enter_context(tc.tile_pool(name="psE", bufs=2, space="PSUM"))
        psB = ctx.enter_context(tc.tile_pool(name="psB", bufs=3, space="PSUM"))

        def wtile(shape, tag, dt=FP):
            return wres.tile(shape, dt, tag=tag, name=tag)

        def loadw(dst, src):
            nc.gpsimd.dma_start(out=dst, in_=src)

        def loads(dst, src):
            nc.sync.dma_start(out=dst, in_=src)

        def body():
            # ===== DMA queue S (sync): start-critical, then history =====
            wemb = wtile([P, KE, HID], "wemb", BF)
            loads(wemb, d_Wemb.rearrange("p (k h) -> p k h", k=KE))
            vct = wtile([P, KV, BC], "vct", BF)
            loads(vct, d_vcT.rearrange("p (k b) -> p k b", k=KV))
            reps_sb = []
            for b in range(BC):
                t = repsp.tile([P, KE, lps[b]], BF, tag="reps", name=f"reps{b}")
                loads(t, d_reps[b][:, :KE * lps[b]]
                      .rearrange("p (k l) -> p k l", k=KE))
                reps_sb.append(t)
            mrows = []
            for b in range(BC):
                m = wtile([1, lps[b]], f"mrow{b}")
                loads(m, d_mask[b:b + 1, :lps[b]])
                mrows.append(m)
            hh_sb = wtile([BS, 1], "hh")
            loads(hh_sb, d_hh[:, :])
            # S tail: the last 8 W_vis groups ride the otherwise-idle sync
            # queue so the full W_vis lands ~25us earlier
            wv2_tiles = {}
            for g in range(16, NVG):
                wv = wvp2.tile([P, WVB, HID], BF, tag="wv2", name="wv2")
                loads(wv, d_Wvis[g].rearrange("p (k h) -> p k h", k=WVB))
                wv2_tiles[g] = wv
            histf_sb = wtile([P, KBH, EMBED], "histf", BF)
            loads(histf_sb, d_hist.rearrange("p (k e) -> p k e", k=KBH))
            validW_sb = wtile([P, KBH, BS], "validW", BF)
            loads(validW_sb, d_validW.rearrange("p (k s) -> p k s", k=KBH))

            # ===== DMA queue W (gpsimd): consts, W_vis stream, weights =====
            bembc_sb = wtile([P, NHT], "bembc")
            loadw(bembc_sb, d_bemb_col[:, :])
            ones = wtile([1, P], "ones", BF)
            loadw(ones, d_ones[:, :])
            ident = wtile([P, P], "ident")
            loadw(ident, d_ident[:, :])
            gsel = wtile([P, BC], "gsel", BF)
            loadw(gsel, d_gsel[:, :])
            bvis_sb = wtile([1, HID], "bvis", BF)
            loadw(bvis_sb, d_bvis[:, :])
            bsep_sb = wtile([1, HID], "bsep", BF)
            loadw(bsep_sb, d_bsep[:, :])
            bembr_sb = wtile([1, HID], "bembr", BF)
            loadw(bembr_sb, d_bemb_row[:, :])
            ba1c_sb = wtile([P, NAT], "ba1c")
            loadw(ba1c_sb, d_ba1_col[:, :])
            bmmc_sb = wtile([P, NHT], "bmmc")
            loadw(bmmc_sb, d_bmm_col[:, :])
            wa2_sb = wtile([P, KA], "wa2", BF)
            loadw(wa2_sb, d_Wa2[:, :])
            wv_tiles = []
            for g in range(16):
                wv = wvp.tile([P, WVB, HID], BF, tag="wv", name="wv")
                loadw(wv, d_Wvis[g].rearrange("p (k h) -> p k h", k=WVB))
                wv_tiles.append(wv)
            for g in range(16, NVG):
                wv_tiles.append(wv2_tiles[g])
            wmm = wtile([P, 2 * KH, HID], "wmm", BF)
            loadw(wmm, d_Wmm.rearrange("p (k h) -> p k h", k=2 * KH))
            wa1 = wtile([P, KH, ATT], "wa1", BF)
            loadw(wa1, d_Wa1.rearrange("p (k h) -> p k h", k=KH))
            # W tail: sep data, consumed between early per-b blocks
            wsep_sb = wtile([P, KI, HID], "wsep", BF)
            loadw(wsep_sb, d_Wsep.rearrange("p (k h) -> p k h", k=KI))
            sepT_sb = wtile([P, KI, BS], "sepT", BF)
            loadw(sepT_sb, d_sepT.rearrange("p (k s) -> p k s", k=KI))

            # =========== phase 1: mm1 for all b, vc interleaved ===========
            vc_ps = psV.tile([P, HID], FP, tag="V", name="vc_ps")
            nc.vector.memset(vc_ps[:, :], 0.0)

            def emit_vc_group(g):
                for j in range(WVB):
                    k = g * WVB + j
                    cg = 32 * j
                    nc.tensor.matmul(vc_ps[cg:cg + BC, :], vct[:, k, :],
                                     wv_tiles[g][:, j, :],
                                     start=(g == 0), stop=(g == NVG - 1),
                                     tile_position=(0, cg),
                                     skip_group_check=True)

            mm1_sb = {}
            vc_emitted = 0
            htile_no = 0

            def maybe_vc():
                # emit each group just after its modeled DMA arrival; the
                # first few groups land before reps b1 does, so they also
                # fill the early DMA-ramp PE gaps
                nonlocal vc_emitted
                while (vc_emitted < 16
                       and htile_no >= min(31, 6 + int(1.3 * vc_emitted))):
                    emit_vc_group(vc_emitted)
                    vc_emitted += 1

            for b in range(BC):
                for h in range(NHT):
                    ps = psM.tile([P, lps[b]], FP, tag="M", name="mm1ps")
                    for k in range(KE):
                        nc.tensor.matmul(
                            ps[:, :],
                            wemb[:, k, h * P:(h + 1) * P],
                            reps_sb[b][:, k, :],
                            start=(k == 0), stop=(k == KE - 1))
                    t = mm1p.tile([P, lps[b]], BF, tag="mm1",
                                  name=f"mm1_{b}_{h}")
                    nc.scalar.activation(t, ps[:, :], AFT.Relu,
                                         bias=bembc_sb[:, h:h + 1])
                    mm1_sb[(b, h)] = t
                    htile_no += 1
                    maybe_vc()
            # =========== phase 2: per-b pipeline with fillers ===========
            ctxmmb_sb = [wtile([P, BC], f"ctxmmb{h}") for h in range(NHT)]

            def emit_ctx():
                vcp_sb = wtile([P, HID], "vcp_sb", BF)
                nc.scalar.activation(vcp_sb, vc_ps[:, :], AFT.Identity)
                ctx_ps = psE.tile([BC, HID], FP, tag="E", name="ctx_ps")
                nc.tensor.matmul(ctx_ps[:, :], gsel[:, :], vcp_sb[:, :],
                                 start=True, stop=False)
                nc.tensor.matmul(ctx_ps[:, :], ones[:, :BC], bvis_sb[:, :],
                                 start=False, stop=True)
                ctx_sb = wtile([BC, HID], "ctx_sb")
                nc.scalar.activation(ctx_sb, ctx_ps[:, :], AFT.Relu)
                ctxT_sb = [wtile([P, BC], f"ctxT{h}", BF) for h in range(NHT)]
                for h in range(NHT):
                    tp = psE.tile([P, BC], FP, tag="E", name="ctxT_ps")
                    nc.tensor.transpose(tp[:, :],
                                        ctx_sb[:, h * P:(h + 1) * P],
                                        ident[:BC, :BC])
                    nc.scalar.activation(ctxT_sb[h], tp[:, :], AFT.Identity)
                for h2 in range(NHT):
                    ps = psE.tile([P, BC], FP, tag="E", name="ctxmm_ps")
                    for k in range(KH):
                        nc.tensor.matmul(ps[:, :],
                                         wmm[:, KH + k, h2 * P:(h2 + 1) * P],
                                         ctxT_sb[k][:, :],
                                         start=(k == 0), stop=(k == KH - 1))
                    nc.scalar.activation(ctxmmb_sb[h2], ps[:, :],
                                         AFT.Identity,
                                         bias=bmmc_sb[:, h2:h2 + 1])

            outrow = wtile([1, BS], "outrow")
            wrow_q = {}
            mm2_q = {}
            sep_sb = wtile([BS, HID], "sep_sb")
            hadd_sb = wtile([BS, HID], "hadd_sb")
            sepfin_sb = wtile([BS, HID], "sepfin_sb")
            sepfinT = [wtile([P, BS], f"sepfinT{h}", BF) for h in range(NHT)]
            havgT_sb = [wtile([P, BS], f"havgT{e}", BF) for e in range(KE)]

            def fill_sep():
                sep_ps = psE.tile([BS, HID], FP, tag="E", name="sep_ps")
                for k in range(KI):
                    nc.tensor.matmul(sep_ps[:, :], sepT_sb[:, k, :],
                                     wsep_sb[:, k, :],
                                     start=(k == 0), stop=False)
                nc.tensor.matmul(sep_ps[:, :], ones[:, :BS], bsep_sb[:, :],
                                 start=False, stop=True)
                nc.vector.tensor_copy(sep_sb, sep_ps[:, :])

            def fill_havg():
                for e in range(KE):
                    ps = psE.tile([P, BS], FP, tag="E", name="havg_ps")
                    for k in range(KBH):
                        nc.tensor.matmul(ps[:, :],
                                         histf_sb[:, k, e * P:(e + 1) * P],
                                         validW_sb[:, k, :],
                                         start=(k == 0), stop=(k == KBH - 1))
                    nc.scalar.activation(havgT_sb[e], ps[:, :], AFT.Identity)

            def fill_ha():
                ha_ps = psE.tile([BS, HID], FP, tag="E", name="ha_ps")
                for e in range(KE):
                    nc.tensor.matmul(ha_ps[:, :], havgT_sb[e][:, :],
                                     wemb[:, e, :],
                                     start=(e == 0), stop=False)
                nc.tensor.matmul(ha_ps[:, :], ones[:, :BS], bembr_sb[:, :],
                                 start=False, stop=True)
                nc.scalar.activation(hadd_sb, ha_ps[:, :], AFT.Relu)

            def fill_sepfin():
                nc.vector.tensor_scalar_mul(sepfin_sb, hadd_sb, hh_sb)
                nc.vector.tensor_add(sepfin_sb, sepfin_sb, sep_sb)
                for h in range(NHT):
                    tp = psE.tile([P, BS], FP, tag="E", name="sfT_ps")
                    nc.tensor.transpose(tp[:, :],
                                        sepfin_sb[:, h * P:(h + 1) * P],
                                        ident[:BS, :BS])
                    nc.scalar.activation(sepfinT[h], tp[:, :], AFT.Identity)

            attc_q = {}

            def attend_p1(b):
                # broadcast softmax row + weighted-sum DVE chain; the PE
                # only issues one matmul here, the DVE work completes while
                # the next per-b block streams
                lp = lps[b]
                wb_ps = psB.tile([P, lp], FP, tag="B", name="wbps")
                nc.tensor.matmul(wb_ps[:, :], ones[:, :], wrow_q.pop(b)[:, :],
                                 start=True, stop=True)
                wbt = wbtp.tile([P, lp], BF, tag="wbt", name="wbt")
                nc.vector.tensor_copy(wbt, wb_ps[:, :])
                attc = []
                for h2 in range(NHT):
                    tmp = tmpp.tile([P, lp], BF, tag="tmpa", name="tmpa")
                    nc.vector.tensor_mul(tmp, mm2_q[b][h2][:, :], wbt)
                    ac = attcp.tile([P, 1], BF, tag="attc", name="attc")
                    with nc.allow_low_precision(
                            reason="attended col consumed by bf16 matmul"):
                        nc.vector.reduce_sum(ac, tmp, axis=AX.X)
                    attc.append(ac)
                del mm2_q[b]
                attc_q[b] = attc

            def attend_p2(b):
                attc = attc_q.pop(b)
                o_ps = psE.tile([1, BS], FP, tag="E", name="o_ps")
                for h2 in range(NHT):
                    nc.tensor.matmul(o_ps[:, :], attc[h2][:, :],
                                     sepfinT[h2][:, :],
                                     start=(h2 == 0), stop=(h2 == NHT - 1))
                nc.vector.tensor_copy(outrow[0:1, S * b:S * (b + 1)],
                                      o_ps[0:1, S * b:S * (b + 1)])

            def emit_block(b):
                lp = lps[b]
                mm2t = []
                for h2 in range(NHT):
                    ps = psB.tile([P, lp], FP, tag="B", name="mm2ps")
                    for k in range(KH):
                        nc.tensor.matmul(ps[:, :],
                                         wmm[:, k, h2 * P:(h2 + 1) * P],
                                         mm1_sb[(b, k)][:, :],
                                         start=(k == 0), stop=(k == KH - 1))
                    t = mm2p.tile([P, lp], BF, tag="mm2", name="mm2t")
                    nc.scalar.activation(t, ps[:, :], AFT.Relu,
                                         bias=ctxmmb_sb[h2][:, b:b + 1])
                    mm2t.append(t)
                mm2_q[b] = mm2t
                atth = []
                for a in range(NAT):
                    ps = psB.tile([P, lp], FP, tag="B", name="mm3ps")
                    for k in range(KH):
                        nc.tensor.matmul(ps[:, :],
                                         wa1[:, k, a * P:(a + 1) * P],
                                         mm2t[k][:, :],
                                         start=(k == 0), stop=(k == KH - 1))
                    t = atthp.tile([P, lp], BF, tag="atth", name="atht")
                    nc.scalar.activation(t, ps[:, :], AFT.Tanh,
                                         bias=ba1c_sb[:, a:a + 1])
                    atth.append(t)
                sc_ps = psB.tile([1, lp], FP, tag="B", name="scps")
                for k in range(KA):
                    nc.tensor.matmul(sc_ps[:, :], wa2_sb[:, k:k + 1],
                                     atth[k][:, :],
                                     start=(k == 0), stop=(k == KA - 1))
                att_row = smp.tile([1, lp], FP, tag="attrow", name="att_row")
                nc.vector.tensor_add(att_row, sc_ps[:, :], mrows[b])
                negmax = smp.tile([1, 1], FP, tag="negmax", name="negmax")
                nc.vector.reduce_max(negmax, att_row, axis=AX.X, negate=True)
                esum = smp.tile([1, 1], FP, tag="esum", name="esum")
                nc.scalar.activation(att_row, att_row, AFT.Exp, bias=negmax,
                                     accum_out=esum)
                rec = smp.tile([1, 1], FP, tag="rec", name="rec")
                nc.vector.reciprocal(rec, esum)
                wrow = wrp.tile([1, lp], BF, tag="wrow", name="wrow")
                nc.scalar.activation(wrow, att_row, AFT.Copy, scale=rec)
                wrow_q[b] = wrow

            # history fillers use S-queue data that lands before the
            # W_vis tail; they occupy the PE while W_vis/W_mm finish
            fill_havg()
            fill_ha()
            while vc_emitted < NVG:
                emit_vc_group(vc_emitted)
                vc_emitted += 1
            emit_ctx()
            emit_block(0)
            emit_block(1)
            emit_block(2)
            fill_sep()
            fill_sepfin()
            emit_block(3)
            attend_p1(0)
            attend_p2(0)
            emit_block(4)
            attend_p1(1)
            attend_p2(1)
            emit_block(5)
            attend_p1(2)
            attend_p2(2)
            attend_p1(3)
            attend_p2(3)
            emit_block(6)
            attend_p1(4)
            attend_p2(4)
            attend_p1(5)
            attend_p2(5)
            emit_block(7)
            attend_p1(6)
            attend_p2(6)
            attend_p1(7)
            attend_p2(7)

            nc.sync.dma_start(out=d_out[:, :], in_=outrow)

        body()

    nc.compile()
    return nc


_NC_CACHE = {}


def kernel(reps, separate_imgs, visual_context, masks, hist, hist_len,
           W_vis, b_vis, W_emb, b_emb, W_mm, b_mm, W_sep, b_sep,
           W_a1, b_a1, W_a2, b_a2):
    f32 = np.float32
    bf16 = ml_dtypes.bfloat16

    def pm(a, kchunks):
        """[K, W] -> partition-major bf16 [128, kchunks*W]."""
        a = np.ascontiguousarray(a, f32)
        K, W = a.shape
        assert K == kchunks * P
        out = a.reshape(kchunks, P, W).transpose(1, 0, 2)
        return np.ascontiguousarray(out).astype(bf16).reshape(P, kchunks * W)

    reps = np.asarray(reps, f32)
    separate_imgs = np.asarray(separate_imgs, f32)
    visual_context = np.asarray(visual_context, f32)
    hist = np.asarray(hist, f32)
    hist_len = np.asarray(hist_len, np.int32)
    masks = np.asarray(masks)[:, :, 0]          # True -> masked out

    # ---- compact each batch's sequence to its unmasked positions ----
    # all cores run one SPMD program, so slot b's capacity is the max
    # keep-count over cores at that position (rounded up to 8)
    keep_idx = [np.nonzero(~masks[b])[0] for b in range(B)]
    prog_lps = tuple(
        min(max((max(len(keep_idx[c * BC + b]) for c in range(NCORES))
                 + 7) // 8 * 8, 8), L)
        for b in range(BC))
    lmax_all = max(prog_lps)

    ident = np.eye(P, dtype=f32)
    gsel = np.zeros((P, BC), f32)
    for j in range(4):
        for i in range(BC):
            gsel[32 * j + i, i] = 1.0

    wvis_pm = np.ascontiguousarray(
        np.asarray(W_vis, f32).reshape(NVG, WVB, P, HID).transpose(0, 2, 1, 3)
    ).astype(bf16).reshape(NVG, P, WVB * HID)

    shared = {
        "Wvis": wvis_pm,
        "Wemb": pm(np.asarray(W_emb, f32), KE),
        "Wmm": pm(np.asarray(W_mm, f32), 2 * KH),
        "Wsep": pm(np.asarray(W_sep, f32), KI),
        "Wa1": pm(np.asarray(W_a1, f32), KH),
        "Wa2": pm(np.asarray(W_a2, f32).reshape(ATT, 1), KA).reshape(P, KA),
        "bvis_row": np.asarray(b_vis, f32).reshape(1, HID).astype(bf16),
        "bsep_row": np.asarray(b_sep, f32).reshape(1, HID).astype(bf16),
        "bemb_row": np.asarray(b_emb, f32).reshape(1, HID).astype(bf16),
        "bemb_col": np.ascontiguousarray(
            np.asarray(b_emb, f32).reshape(NHT, P).T),
        "bmm_col": np.ascontiguousarray(
            np.asarray(b_mm, f32).reshape(NHT, P).T),
        "ba1_col": np.ascontiguousarray(
            np.asarray(b_a1, f32).reshape(NAT, P).T),
        "ones_row": np.ones((1, P), bf16),
        "ident": ident,
        "gsel": gsel.astype(bf16),
    }

    in_maps = []
    for c in range(NCORES):
        sl = slice(c * BC, (c + 1) * BC)
        repsT = np.zeros((BC, P, KE * lmax_all), bf16)
        mask_c = np.zeros((BC, lmax_all), f32)
        for b in range(BC):
            gb = c * BC + b
            ix = keep_idx[gb]
            lp = prog_lps[b]
            r = np.zeros((lp, EMBED), f32)
            r[:len(ix)] = reps[gb, ix]
            rpm = r.reshape(lp, KE, P).transpose(2, 1, 0)  # [P, KE, lp]
            repsT[b, :, :KE * lp] = np.ascontiguousarray(rpm) \
                .astype(bf16).reshape(P, KE * lp)
            mask_c[b, :lp] = f32(-1e30)
            mask_c[b, :len(ix)] = 0.0
        mask_c += f32(b_a2[0])

        hl = hist_len[sl].reshape(BS)
        hvalid = (np.arange(H)[None, :] < hl[:, None]).astype(f32)
        hvalid /= np.maximum(hl, 1).astype(f32)[:, None]
        validW = np.zeros((BSH, BS), f32)
        for bs in range(BS):
            validW[bs * H:(bs + 1) * H, bs] = hvalid[bs]
        vcT = visual_context[sl].reshape(BC, KV, P).transpose(2, 1, 0)
        sepT = separate_imgs[sl].reshape(BS, KI, P).transpose(2, 1, 0)
        m = {
            "repsT": repsT,
            "vcT": np.ascontiguousarray(vcT).astype(bf16).reshape(P, KV * BC),
            "sepT": np.ascontiguousarray(sepT).astype(bf16)
                      .reshape(P, KI * BS),
            "histf": pm(hist[sl].reshape(BSH, EMBED), KBH),
            "validW": pm(validW, KBH),
            "mask_row": mask_c,
            "hh_col": (hl > 0).astype(f32).reshape(BS, 1),
        }
        m.update(shared)
        in_maps.append(m)

    if prog_lps not in _NC_CACHE:
        _NC_CACHE[prog_lps] = build_nc(prog_lps)
    res = run_bass_kernel_spmd(_NC_CACHE[prog_lps], in_maps,
                               list(range(NCORES)))
    out = np.concatenate([r["out"].reshape(BC, S, 1) for r in res.results],
                         axis=0)
    return out.astype(f32)


if __name__ == "__main__":
    pass


# revision 46
# speedup vs baseline: 1.0804x; 1.0804x over previous
"""Trainium2 Bass kernel for nn_ListenerModel (scatter_memory).

Strategy: pure data-parallel over batch (B=64 -> 8 rows/core), weights
replicated.  Key points:
  - masked sequence positions have softmax weight exactly 0, so the
    host compacts each batch's sequence to its unmasked positions
    (variable per-batch length lp_b, rounded to 8).  Math is exact;
    pad slots get -1e30 mask.  This cuts the dominant matmul chain and
    reps DMA ~2x.
  - all matmul operands bf16 (fp32r streams 2 cyc/col; bf16 1).
  - host lays every tensor out partition-major so DMAs are contiguous;
    two queues (sync: start-critical + activations, gpsimd: W_vis
    stream + late weights) ordered by need-time; reps pool holds all
    8 batches so no DMA ever waits on compute.
  - visual-context matmuls (M=8) packed 4-wide into PE column groups,
    interleaved into mm1 against modeled DMA arrival times.
  - per-b softmax/attend runs several blocks behind the matmul stream;
    sep/history fillers are interleaved between early per-b blocks
    (their data arrives on the queue tails); outputs produced per-b
    via attcol x sepfinT matmuls (no serial transpose tail).
"""

import numpy as np
import ml_dtypes
from contextlib import ExitStack

import concourse.bass as bass
import concourse.mybir as mybir
from concourse import bacc, tile
from concourse.bass_utils import run_bass_kernel_spmd

NCORES = 8
B, L, S, H = 64, 512, 6, 8
EMBED, HID, IMG, ATT = 1024, 512, 2048, 256
SIMG = S * IMG          # 12288
BC = B // NCORES        # 8 batch rows per core
BS = BC * S             # 48 (b,s) rows per core
BSH = BS * H            # 384
P = 128
FP = mybir.dt.float32
BF = mybir.dt.bfloat16

KE = EMBED // P         # 8  k-chunks for EMBED contraction
KH = HID // P           # 4  k-chunks for HID contraction
KA = ATT // P           # 2  k-chunks for ATT contraction
KV = SIMG // P          # 96 k-chunks for the visual-context matmul
KI = IMG // P           # 16 k-chunks for separate-image projection
KBH = BSH // P          # 3  k-chunks for history averaging
NHT = HID // P          # 4  hid tiles
NAT = ATT // P          # 2  att tiles

WVB = 4                 # W_vis chunks per DMA / per packed vc group
NVG = KV // WVB         # 24 vc chunk groups


def build_nc(lps):
    """lps: per-core tuple of BC compacted sequence lengths."""
    lmax = max(lps)
    nc = bacc.Bacc(None)

    d_reps = nc.dram_tensor("repsT", [BC, P, KE * lmax], BF,
                            kind="ExternalInput")
    d_vcT = nc.dram_tensor("vcT", [P, KV * BC], BF, kind="ExternalInput")
    d_sepT = nc.dram_tensor("sepT", [P, KI * BS], BF, kind="ExternalInput")
    d_hist = nc.dram_tensor("histf", [P, KBH * EMBED], BF, kind="ExternalInput")
    d_validW = nc.dram_tensor("validW", [P, KBH * BS], BF, kind="ExternalInput")
    d_Wvis = nc.dram_tensor("Wvis", [NVG, P, WVB * HID], BF, kind="ExternalInput")
    d_Wemb = nc.dram_tensor("Wemb", [P, KE * HID], BF, kind="ExternalInput")
    d_Wmm = nc.dram_tensor("Wmm", [P, 2 * KH * HID], BF, kind="ExternalInput")
    d_Wsep = nc.dram_tensor("Wsep", [P, KI * HID], BF, kind="ExternalInput")
    d_Wa1 = nc.dram_tensor("Wa1", [P, KH * ATT], BF, kind="ExternalInput")
    d_Wa2 = nc.dram_tensor("Wa2", [P, KA], BF, kind="ExternalInput")
    d_bvis = nc.dram_tensor("bvis_row", [1, HID], BF, kind="ExternalInput")
    d_bsep = nc.dram_tensor("bsep_row", [1, HID], BF, kind="ExternalInput")
    d_bemb_row = nc.dram_tensor("bemb_row", [1, HID], BF, kind="ExternalInput")
    d_ones = nc.dram_tensor("ones_row", [1, P], BF, kind="ExternalInput")
    d_bemb_col = nc.dram_tensor("bemb_col", [P, NHT], FP, kind="ExternalInput")
    d_bmm_col = nc.dram_tensor("bmm_col", [P, NHT], FP, kind="ExternalInput")
    d_ba1_col = nc.dram_tensor("ba1_col", [P, NAT], FP, kind="ExternalInput")
    d_mask = nc.dram_tensor("mask_row", [BC, lmax], FP, kind="ExternalInput")
    d_hh = nc.dram_tensor("hh_col", [BS, 1], FP, kind="ExternalInput")
    d_ident = nc.dram_tensor("ident", [P, P], FP, kind="ExternalInput")
    d_gsel = nc.dram_tensor("gsel", [P, BC], BF, kind="ExternalInput")
    d_out = nc.dram_tensor("out", [1, BS], FP, kind="ExternalOutput")

    AFT = mybir.ActivationFunctionType
    AX = mybir.AxisListType

    with ExitStack() as ctx:
        tc = ctx.enter_context(tile.TileContext(nc))
        wres = ctx.enter_context(tc.tile_pool(name="wres", bufs=1))
        repsp = ctx.enter_context(tc.tile_pool(name="repsp", bufs=8))
        wvp = ctx.enter_context(tc.tile_pool(name="wvp", bufs=4))
        mm1p = ctx.enter_context(tc.tile_pool(name="mm1p", bufs=32))
        mm2p = ctx.enter_context(tc.tile_pool(name="mm2p", bufs=20))
        atthp = ctx.enter_context(tc.tile_pool(name="atthp", bufs=4))
        tmpp = ctx.enter_context(tc.tile_pool(name="tmpp", bufs=3))
        wbtp = ctx.enter_context(tc.tile_pool(name="wbtp", bufs=2))
        attcp = ctx.enter_context(tc.tile_pool(name="attcp", bufs=32))
        smp = ctx.enter_context(tc.tile_pool(name="smp", bufs=3))
        wrp = ctx.enter_context(tc.tile_pool(name="wrp", bufs=8))
        # PSUM: 2 + 1 + 2 + 3 = 8 banks
        psM = ctx.enter_context(tc.tile_pool(name="psM", bufs=2, space="PSUM"))
        psV = ctx.enter_context(tc.tile_pool(name="psV", bufs=1, space="PSUM"))
        psE = ctx.enter_context(tc.tile_pool(name="psE", bufs=1, space="PSUM"))
        psB = ctx.enter_context(tc.tile_pool(name="psB", bufs=4, space="PSUM"))

        def wtile(shape, tag, dt=FP):
            return wres.tile(shape, dt, tag=tag, name=tag)

        def loadw(dst, src):
            nc.gpsimd.dma_start(out=dst, in_=src)

        def loads(dst, src):
            nc.sync.dma_start(out=dst, in_=src)

        def body():
            # ===== DMA queue S (sync): start-critical, then history =====
            wemb = wtile([P, KE, HID], "wemb", BF)
            loads(wemb, d_Wemb.rearrange("p (k h) -> p k h", k=KE))
            reps_sb = []
            for b in range(BC):
                t = repsp.tile([P, KE, lps[b]], BF, tag="reps", name=f"reps{b}")
                loads(t, d_reps[b][:, :KE * lps[b]]
                      .rearrange("p (k l) -> p k l", k=KE))
                reps_sb.append(t)
                if b == 1:
                    vct = wtile([P, KV, BC], "vct", BF)
                    loads(vct, d_vcT.rearrange("p (k b) -> p k b", k=KV))
            mrows = []
            for b in range(BC):
                m = wtile([1, lps[b]], f"mrow{b}")
                loads(m, d_mask[b:b + 1, :lps[b]])
                mrows.append(m)
            hh_sb = wtile([BS, 1], "hh")
            loads(hh_sb, d_hh[:, :])
            # S tail: history data (fills the pre-ctx PE window)
            histf_sb = wtile([P, KBH, EMBED], "histf", BF)
            loads(histf_sb, d_hist.rearrange("p (k e) -> p k e", k=KBH))
            validW_sb = wtile([P, KBH, BS], "validW", BF)
            loads(validW_sb, d_validW.rearrange("p (k s) -> p k s", k=KBH))

            # ===== DMA queue W (gpsimd): consts, W_vis stream, weights =====
            bembc_sb = wtile([P, NHT], "bembc")
            loadw(bembc_sb, d_bemb_col[:, :])
            ones = wtile([1, P], "ones", BF)
            loadw(ones, d_ones[:, :])
            ident = wtile([P, P], "ident")
            loadw(ident, d_ident[:, :])
            gsel = wtile([P, BC], "gsel", BF)
            loadw(gsel, d_gsel[:, :])
            bvis_sb = wtile([1, HID], "bvis", BF)
            loadw(bvis_sb, d_bvis[:, :])
            bsep_sb = wtile([1, HID], "bsep", BF)
            loadw(bsep_sb, d_bsep[:, :])
            bembr_sb = wtile([1, HID], "bembr", BF)
            loadw(bembr_sb, d_bemb_row[:, :])
            ba1c_sb = wtile([P, NAT], "ba1c")
            loadw(ba1c_sb, d_ba1_col[:, :])
            bmmc_sb = wtile([P, NHT], "bmmc")
            loadw(bmmc_sb, d_bmm_col[:, :])
            wa2_sb = wtile([P, KA], "wa2", BF)
            loadw(wa2_sb, d_Wa2[:, :])
            wv_tiles = []
            for g in range(NVG):
                wv = wvp.tile([P, WVB, HID], BF, tag="wv", name="wv")
                loadw(wv, d_Wvis[g].rearrange("p (k h) -> p k h", k=WVB))
                wv_tiles.append(wv)
            wmm = wtile([P, 2 * KH, HID], "wmm", BF)
            loadw(wmm, d_Wmm.rearrange("p (k h) -> p k h", k=2 * KH))
            wa1 = wtile([P, KH, ATT], "wa1", BF)
            loadw(wa1, d_Wa1.rearrange("p (k h) -> p k h", k=KH))
            # W tail: sep data, consumed between early per-b blocks
            wsep_sb = wtile([P, KI, HID], "wsep", BF)
            loadw(wsep_sb, d_Wsep.rearrange("p (k h) -> p k h", k=KI))
            sepT_sb = wtile([P, KI, BS], "sepT", BF)
            loadw(sepT_sb, d_sepT.rearrange("p (k s) -> p k s", k=KI))

            # =========== phase 1: mm1 for all b, vc interleaved ===========
            vc_ps = psV.tile([P, HID], FP, tag="V", name="vc_ps")
            nc.vector.memset(vc_ps[:, :], 0.0)

            def emit_vc_group(g):
                for j in range(WVB):
                    k = g * WVB + j
                    cg = 32 * j
                    nc.tensor.matmul(vc_ps[cg:cg + BC, :], vct[:, k, :],
                                     wv_tiles[g][:, j, :],
                                     start=(g == 0), stop=(g == NVG - 1),
                                     tile_position=(0, cg),
                                     skip_group_check=True)

            mm1_sb = {}
            vc_emitted = 0
            htile_no = 0

            def maybe_vc():
                # emit each group just after its modeled DMA arrival; the
                # first few groups land before reps b1 does, so they also
                # fill the early DMA-ramp PE gaps
                nonlocal vc_emitted
                while (vc_emitted < 18
                       and htile_no >= min(31, 6 + int(1.4 * vc_emitted))):
                    emit_vc_group(vc_emitted)
                    vc_emitted += 1

            for b in range(BC):
                for h in range(NHT):
                    ps = psM.tile([P, lps[b]], FP, tag="M", name="mm1ps")
                    for k in range(KE):
                        nc.tensor.matmul(
                            ps[:, :],
                            wemb[:, k, h * P:(h + 1) * P],
                            reps_sb[b][:, k, :],
                            start=(k == 0), stop=(k == KE - 1))
                    t = mm1p.tile([P, lps[b]], BF, tag="mm1",
                                  name=f"mm1_{b}_{h}")
                    nc.scalar.activation(t, ps[:, :], AFT.Relu,
                                         bias=bembc_sb[:, h:h + 1])
                    mm1_sb[(b, h)] = t
                    htile_no += 1
                    maybe_vc()
            # =========== phase 2: per-b pipeline with fillers ===========
            ctxmmb_sb = [wtile([P, BC], f"ctxmmb{h}") for h in range(NHT)]

            def emit_ctx():
                vcp_sb = wtile([P, HID], "vcp_sb", BF)
                nc.scalar.activation(vcp_sb, vc_ps[:, :], AFT.Identity)
                ctx_ps = psE.tile([BC, HID], FP, tag="E", name="ctx_ps")
                nc.tensor.matmul(ctx_ps[:, :], gsel[:, :], vcp_sb[:, :],
                                 start=True, stop=False)
                nc.tensor.matmul(ctx_ps[:, :], ones[:, :BC], bvis_sb[:, :],
                                 start=False, stop=True)
                ctx_sb = wtile([BC, HID], "ctx_sb")
                nc.scalar.activation(ctx_sb, ctx_ps[:, :], AFT.Relu)
                ctxT_sb = [wtile([P, BC], f"ctxT{h}", BF) for h in range(NHT)]
                for h in range(NHT):
                    tp = psE.tile([P, BC], FP, tag="E", name="ctxT_ps")
                    nc.tensor.transpose(tp[:, :],
                                        ctx_sb[:, h * P:(h + 1) * P],
                                        ident[:BC, :BC])
                    nc.scalar.activation(ctxT_sb[h], tp[:, :], AFT.Identity)
                for h2 in range(NHT):
                    ps = psE.tile([P, BC], FP, tag="E", name="ctxmm_ps")
                    for k in range(KH):
                        nc.tensor.matmul(ps[:, :],
                                         wmm[:, KH + k, h2 * P:(h2 + 1) * P],
                                         ctxT_sb[k][:, :],
                                         start=(k == 0), stop=(k == KH - 1))
                    nc.scalar.activation(ctxmmb_sb[h2], ps[:, :],
                                         AFT.Identity,
                                         bias=bmmc_sb[:, h2:h2 + 1])

            outrow = wtile([1, BS], "outrow")
            wrow_q = {}
            mm2_q = {}
            sep_sb = wtile([BS, HID], "sep_sb")
            hadd_sb = wtile([BS, HID], "hadd_sb")
            sepfin_sb = wtile([BS, HID], "sepfin_sb")
            sepfinT = [wtile([P, BS], f"sepfinT{h}", BF) for h in range(NHT)]
            havgT_sb = [wtile([P, BS], f"havgT{e}", BF) for e in range(KE)]

            def fill_sep():
                sep_ps = psE.tile([BS, HID], FP, tag="E", name="sep_ps")
                for k in range(KI):
                    nc.tensor.matmul(sep_ps[:, :], sepT_sb[:, k, :],
                                     wsep_sb[:, k, :],
                                     start=(k == 0), stop=False)
                nc.tensor.matmul(sep_ps[:, :], ones[:, :BS], bsep_sb[:, :],
                                 start=False, stop=True)
                nc.vector.tensor_copy(sep_sb, sep_ps[:, :])

            def fill_havg():
                for e in range(KE):
                    ps = psE.tile([P, BS], FP, tag="E", name="havg_ps")
                    for k in range(KBH):
                        nc.tensor.matmul(ps[:, :],
                                         histf_sb[:, k, e * P:(e + 1) * P],
                                         validW_sb[:, k, :],
                                         start=(k == 0), stop=(k == KBH - 1))
                    nc.scalar.activation(havgT_sb[e], ps[:, :], AFT.Identity)

            def fill_ha():
                ha_ps = psE.tile([BS, HID], FP, tag="E", name="ha_ps")
                for e in range(KE):
                    nc.tensor.matmul(ha_ps[:, :], havgT_sb[e][:, :],
                                     wemb[:, e, :],
                                     start=(e == 0), stop=False)
                nc.tensor.matmul(ha_ps[:, :], ones[:, :BS], bembr_sb[:, :],
                                 start=False, stop=True)
                nc.scalar.activation(hadd_sb, ha_ps[:, :], AFT.Relu)

            def fill_sepfin():
                nc.vector.tensor_scalar_mul(sepfin_sb, hadd_sb, hh_sb)
                nc.vector.tensor_add(sepfin_sb, sepfin_sb, sep_sb)
                for h in range(NHT):
                    tp = psE.tile([P, BS], FP, tag="E", name="sfT_ps")
                    nc.tensor.transpose(tp[:, :],
                                        sepfin_sb[:, h * P:(h + 1) * P],
                                        ident[:BS, :BS])
                    nc.scalar.activation(sepfinT[h], tp[:, :], AFT.Identity)

            attc_q = {}

            def attend_p1(b):
                # broadcast softmax row + weighted-sum DVE chain; the PE
                # only issues one matmul here, the DVE work completes while
                # the next per-b block streams
                lp = lps[b]
                wb_ps = psB.tile([P, lp], FP, tag="B", name="wbps")
                nc.tensor.matmul(wb_ps[:, :], ones[:, :], wrow_q.pop(b)[:, :],
                                 start=True, stop=True)
                wbt = wbtp.tile([P, lp], BF, tag="wbt", name="wbt")
                nc.vector.tensor_copy(wbt, wb_ps[:, :])
                attc = []
                for h2 in range(NHT):
                    tmp = tmpp.tile([P, lp], BF, tag="tmpa", name="tmpa")
                    nc.vector.tensor_mul(tmp, mm2_q[b][h2][:, :], wbt)
                    ac = attcp.tile([P, 1], BF, tag="attc", name="attc")
                    with nc.allow_low_precision(
                            reason="attended col consumed by bf16 matmul"):
                        nc.vector.reduce_sum(ac, tmp, axis=AX.X)
                    attc.append(ac)
                del mm2_q[b]
                attc_q[b] = attc

            def attend_p2(b):
                attc = attc_q.pop(b)
                o_ps = psE.tile([1, BS], FP, tag="E", name="o_ps")
                for h2 in range(NHT):
                    nc.tensor.matmul(o_ps[:, :], attc[h2][:, :],
                                     sepfinT[h2][:, :],
                                     start=(h2 == 0), stop=(h2 == NHT - 1))
                nc.vector.tensor_copy(outrow[0:1, S * b:S * (b + 1)],
                                      o_ps[0:1, S * b:S * (b + 1)])

            def emit_block(b):
                lp = lps[b]
                mm2t = []
                for h2 in range(NHT):
                    ps = psB.tile([P, lp], FP, tag="B", name="mm2ps")
                    for k in range(KH):
                        nc.tensor.matmul(ps[:, :],
                                         wmm[:, k, h2 * P:(h2 + 1) * P],
                                         mm1_sb[(b, k)][:, :],
                                         start=(k == 0), stop=(k == KH - 1))
                    t = mm2p.tile([P, lp], BF, tag="mm2", name="mm2t")
                    nc.scalar.activation(t, ps[:, :], AFT.Relu,
                                         bias=ctxmmb_sb[h2][:, b:b + 1])
                    mm2t.append(t)
                mm2_q[b] = mm2t
                atth = []
                for a in range(NAT):
                    ps = psB.tile([P, lp], FP, tag="B", name="mm3ps")
                    for k in range(KH):
                        nc.tensor.matmul(ps[:, :],
                                         wa1[:, k, a * P:(a + 1) * P],
                                         mm2t[k][:, :],
                                         start=(k == 0), stop=(k == KH - 1))
                    t = atthp.tile([P, lp], BF, tag="atth", name="atht")
                    nc.scalar.activation(t, ps[:, :], AFT.Tanh,
                                         bias=ba1c_sb[:, a:a + 1])
                    atth.append(t)
                sc_ps = psB.tile([1, lp], FP, tag="B", name="scps")
                for k in range(KA):
                    nc.tensor.matmul(sc_ps[:, :], wa2_sb[:, k:k + 1],
                                     atth[k][:, :],
                                     start=(k == 0), stop=(k == KA - 1))
                att_row = smp.tile([1, lp], FP, tag="attrow", name="att_row")
                nc.vector.tensor_add(att_row, sc_ps[:, :], mrows[b])
                negmax = smp.tile([1, 1], FP, tag="negmax", name="negmax")
                nc.vector.reduce_max(negmax, att_row, axis=AX.X, negate=True)
                esum = smp.tile([1, 1], FP, tag="esum", name="esum")
                nc.scalar.activation(att_row, att_row, AFT.Exp, bias=negmax,
                                     accum_out=esum)
                rec = smp.tile([1, 1], FP, tag="rec", name="rec")
                nc.vector.reciprocal(rec, esum)
                wrow = wrp.tile([1, lp], BF, tag="wrow", name="wrow")
                nc.scalar.activation(wrow, att_row, AFT.Copy, scale=rec)
                wrow_q[b] = wrow

            # history fillers use S-queue data that lands before the
            # W_vis tail; they occupy the PE while W_vis/W_mm finish
            fill_havg()
            fill_ha()
            while vc_emitted < NVG:
                emit_vc_group(vc_emitted)
                vc_emitted += 1
            emit_ctx()
            emit_block(0)
            emit_block(1)
            emit_block(2)
            fill_sep()
            fill_sepfin()
            emit_block(3)
            attend_p1(0)
            attend_p2(0)
            emit_block(4)
            attend_p1(1)
            attend_p2(1)
            emit_block(5)
            attend_p1(2)
            attend_p2(2)
            attend_p1(3)
            attend_p2(3)
            emit_block(6)
            attend_p1(4)
            attend_p2(4)
            attend_p1(5)
            attend_p2(5)
            emit_block(7)
            attend_p1(6)
            attend_p2(6)
            attend_p1(7)
            attend_p2(7)

            nc.sync.dma_start(out=d_out[:, :], in_=outrow)

        body()

    nc.compile()
    return nc


_NC_CACHE = {}


def kernel(reps, separate_imgs, visual_context, masks, hist, hist_len,
           W_vis, b_vis, W_emb, b_emb, W_mm, b_mm, W_sep, b_sep,
           W_a1, b_a1, W_a2, b_a2):
    f32 = np.float32
    bf16 = ml_dtypes.bfloat16

    def pm(a, kchunks):
        """[K, W] -> partition-major bf16 [128, kchunks*W]."""
        a = np.ascontiguousarray(a, f32)
        K, W = a.shape
        assert K == kchunks * P
        out = a.reshape(kchunks, P, W).transpose(1, 0, 2)
        return np.ascontiguousarray(out).astype(bf16).reshape(P, kchunks * W)

    reps = np.asarray(reps, f32)
    separate_imgs = np.asarray(separate_imgs, f32)
    visual_context = np.asarray(visual_context, f32)
    hist = np.asarray(hist, f32)
    hist_len = np.asarray(hist_len, np.int32)
    masks = np.asarray(masks)[:, :, 0]          # True -> masked out

    # ---- compact each batch's sequence to its unmasked positions ----
    # all cores run one SPMD program, so slot b's capacity is the max
    # keep-count over cores at that position (rounded up to 8)
    keep_idx = [np.nonzero(~masks[b])[0] for b in range(B)]
    prog_lps = tuple(
        min(max((max(len(keep_idx[c * BC + b]) for c in range(NCORES))
                 + 7) // 8 * 8, 8), L)
        for b in range(BC))
    lmax_all = max(prog_lps)

    ident = np.eye(P, dtype=f32)
    gsel = np.zeros((P, BC), f32)
    for j in range(4):
        for i in range(BC):
            gsel[32 * j + i, i] = 1.0

    wvis_pm = np.ascontiguousarray(
        np.asarray(W_vis, f32).reshape(NVG, WVB, P, HID).transpose(0, 2, 1, 3)
    ).astype(bf16).reshape(NVG, P, WVB * HID)

    shared = {
        "Wvis": wvis_pm,
        "Wemb": pm(np.asarray(W_emb, f32), KE),
        "Wmm": pm(np.asarray(W_mm, f32), 2 * KH),
        "Wsep": pm(np.asarray(W_sep, f32), KI),
        "Wa1": pm(np.asarray(W_a1, f32), KH),
        "Wa2": pm(np.asarray(W_a2, f32).reshape(ATT, 1), KA).reshape(P, KA),
        "bvis_row": np.asarray(b_vis, f32).reshape(1, HID).astype(bf16),
        "bsep_row": np.asarray(b_sep, f32).reshape(1, HID).astype(bf16),
        "bemb_row": np.asarray(b_emb, f32).reshape(1, HID).astype(bf16),
        "bemb_col": np.ascontiguousarray(
            np.asarray(b_emb, f32).reshape(NHT, P).T),
        "bmm_col": np.ascontiguousarray(
            np.asarray(b_mm, f32).reshape(NHT, P).T),
        "ba1_col": np.ascontiguousarray(
            np.asarray(b_a1, f32).reshape(NAT, P).T),
        "ones_row": np.ones((1, P), bf16),
        "ident": ident,
        "gsel": gsel.astype(bf16),
    }

    in_maps = []
    for c in range(NCORES):
        sl = slice(c * BC, (c + 1) * BC)
        repsT = np.zeros((BC, P, KE * lmax_all), bf16)
        mask_c = np.zeros((BC, lmax_all), f32)
        for b in range(BC):
            gb = c * BC + b
            ix = keep_idx[gb]
            lp = prog_lps[b]
            r = np.zeros((lp, EMBED), f32)
            r[:len(ix)] = reps[gb, ix]
            rpm = r.reshape(lp, KE, P).transpose(2, 1, 0)  # [P, KE, lp]
            repsT[b, :, :KE * lp] = np.ascontiguousarray(rpm) \
                .astype(bf16).reshape(P, KE * lp)
            mask_c[b, :lp] = f32(-1e30)
            mask_c[b, :len(ix)] = 0.0
        mask_c += f32(b_a2[0])

        hl = hist_len[sl].reshape(BS)
        hvalid = (np.arange(H)[None, :] < hl[:, None]).astype(f32)
        hvalid /= np.maximum(hl, 1).astype(f32)[:, None]
        validW = np.zeros((BSH, BS), f32)
        for bs in range(BS):
            validW[bs * H:(bs + 1) * H, bs] = hvalid[bs]
        vcT = visual_context[sl].reshape(BC, KV, P).transpose(2, 1, 0)
        sepT = separate_imgs[sl].reshape(BS, KI, P).transpose(2, 1, 0)
        m = {
            "repsT": repsT,
            "vcT": np.ascontiguousarray(vcT).astype(bf16).reshape(P, KV * BC),
            "sepT": np.ascontiguousarray(sepT).astype(bf16)
                      .reshape(P, KI * BS),
            "histf": pm(hist[sl].reshape(BSH, EMBED), KBH),
            "validW": pm(validW, KBH),
            "mask_row": mask_c,
            "hh_col": (hl > 0).astype(f32).reshape(BS, 1),
        }
        m.update(shared)
        in_maps.append(m)

    if prog_lps not in _NC_CACHE:
        _NC_CACHE[prog_lps] = build_nc(prog_lps)
    res = run_bass_kernel_spmd(_NC_CACHE[prog_lps], in_maps,
                               list(range(NCORES)))
    out = np.concatenate([r["out"].reshape(BC, S, 1) for r in res.results],
                         axis=0)
    return out.astype(f32)


if __name__ == "__main__":
    pass
